# revision 31
# baseline (speedup 1.0000x reference)
"""Trainium2 Bass kernel for nn_BoundaryHead_contrast (CenterNet-style 1D NMS head).

Strategy (8 NeuronCores, pure data parallel over batch):
  - Device is a pure matvec streamer over a COMPACTED stream: the host keeps
    only positions with saliency >= 0 (masked positions are provably inert:
    their cp is exactly 0, which can never beat an unmasked sigmoid and only
    yields score-0 survivors that cannot reach the top-100 while >= 100
    positive survivors exist - verified, with exact fallback).
  - fp8 mode (default): x is cast to e4m3 (1 B/elem) and packed
    [NSB, 128, 4, 2, 1024] for DoubleRow matmuls (K=256 per pass, 2 fp8
    elements per PE cell per cycle). The three [1024,1] heads ride in one
    [128, 4, 2, 16] fp8 stationary holding 3 heads x 3 e4m3 levels (scaled
    16x per level; host rescales and sums planes -> W exact to ~2^-12).
    HBM traffic: ~17 MiB/core vs 128 MiB f32-equivalent.
  - fp16 mode (KERNEL_F16=1): same pipeline at 2 B/elem without DoubleRow.
  - Host: scatters device center scores back to the full grid, applies the
    mask, runs 5-window NMS + top-k approximately, then refines every
    decision within a conservative margin DELTA by recomputing exact scores
    (f64 dot -> f32, replicating the reference's f32 elementwise ops and tie
    semantics in sigmoid space) for the borderline positions per row
    (~700/row at fp8 margins). Rows where any margin check fails fall back
    to exact host computation of the whole row.
"""

import numpy as np
import ml_dtypes
from contextlib import ExitStack

import concourse.bass as bass
import concourse.tile as tile
from concourse import bacc, mybir
from concourse.bass_utils import run_bass_kernel_spmd

B, L, D = 32, 8192, 1024
NCORES = 8
RPC = B // NCORES          # 4 rows per core
NROW = RPC * L             # 32768 positions per core
NSB = 16                   # full super-blocks of 1024 positions per core
TAIL = 512                 # extra half-super-block
CAP = NSB * 1024 + TAIL    # device capacity; unmasked ~16376 +- 90, max 16480
TOPK = 100
UNIT = 2
EPS = 2.0e-3               # slack so strict logit gaps survive f32 sigmoid

F16, BF16, F32 = mybir.dt.float16, mybir.dt.bfloat16, mybir.dt.float32
F8 = mybir.dt.float8e4
E4M3 = ml_dtypes.float8_e4m3fn

_NC_CACHE = {}


def _build_nc_fp16(nsb):
    nc = bacc.Bacc("TRN2", target_bir_lowering=False, debug=False)
    xpk = nc.dram_tensor("xpk", [nsb, 128, 8, 1024], F16, kind="ExternalInput").ap()
    sta = nc.dram_tensor("sta", [128, 8, 9], BF16, kind="ExternalInput").ap()
    opl = nc.dram_tensor("opl", [9, nsb * 1024], F32, kind="ExternalOutput").ap()
    xpk_v = xpk.rearrange("s k c j -> k s c j")

    with tile.TileContext(nc) as tc, ExitStack() as ctx:
        cpool = ctx.enter_context(tc.tile_pool(name="const", bufs=1))
        xpool = ctx.enter_context(tc.tile_pool(name="xin", bufs=5))
        pspool = ctx.enter_context(tc.tile_pool(name="ps", bufs=3, space="PSUM"))
        evpool = ctx.enter_context(tc.tile_pool(name="ev", bufs=4))

        sta_sb = cpool.tile([128, 8, 9], BF16)
        nc.scalar.dma_start(sta_sb[:], sta)

        for sb in range(nsb):
            xt = xpool.tile([128, 1, 8, 1024], F16, tag="x", name="xt")
            nc.sync.dma_start(xt[:], xpk_v[:, sb:sb + 1, :, :])
            pss = [pspool.tile([9, 512], F32, tag=f"ps{h}", name=f"ps{h}")
                   for h in range(2)]
            for c in range(8):
                for h in range(2):
                    nc.tensor.matmul(pss[h][:, :], sta_sb[:, c, :],
                                     xt[:, 0, c, 512 * h:512 * h + 512],
                                     start=(c == 0), stop=(c == 7),
                                     skip_group_check=True)
            for h in range(2):
                ev = evpool.tile([9, 512], F32, tag=f"ev{h}", name=f"ev{h}")
                nc.scalar.copy(ev[:], pss[h][:])
                o0 = 1024 * sb + 512 * h
                nc.scalar.dma_start(opl[:, o0:o0 + 512], ev[:])

    nc.compile()
    return nc


def _build_nc_fp8(nsb, tail):
    nc = bacc.Bacc("TRN2", target_bir_lowering=False, debug=False)
    xpk = nc.dram_tensor("xpk", [nsb, 128, 4, 2, 1024], F8,
                         kind="ExternalInput").ap()
    xtl = nc.dram_tensor("xtl", [128, 4, 2, tail], F8, kind="ExternalInput").ap()
    sta = nc.dram_tensor("sta", [128, 4, 2, 16], F8, kind="ExternalInput").ap()
    opl = nc.dram_tensor("opl", [3, nsb * 1024 + tail], F32,
                         kind="ExternalOutput").ap()
    xpk_v = xpk.rearrange("s k a i j -> k s a i j")
    DR = mybir.MatmulPerfMode.DoubleRow

    with tile.TileContext(nc) as tc, ExitStack() as ctx:
        cpool = ctx.enter_context(tc.tile_pool(name="const", bufs=1))
        xpool = ctx.enter_context(tc.tile_pool(name="xin", bufs=1))
        pspool = ctx.enter_context(tc.tile_pool(name="ps", bufs=3, space="PSUM"))
        evpool = ctx.enter_context(tc.tile_pool(name="ev", bufs=3))

        sta_sb = cpool.tile([128, 4, 2, 16], F8)
        nc.scalar.dma_start(sta_sb[:], sta)

        def do_sb(xsrc, ev, eo, nh):
            pss = [pspool.tile([16, 512], F32, tag=f"ps{h}", name=f"ps{h}")
                   for h in range(nh)]
            for h in range(nh):
                for a in range(4):
                    nc.tensor.matmul(pss[h][:, :], sta_sb[:, a, :, :],
                                     xsrc(a, h),
                                     start=(a == 0), stop=(a == 3),
                                     perf_mode=DR, skip_group_check=True)
            for h in range(nh):
                nc.scalar.copy(ev[:, eo + 512 * h:eo + 512 * h + 512],
                               pss[h][0:3, :])

        # outputs: ONE DMA per input group, on the gpsimd ring - it carries
        # nothing MM- or input-critical, so an output's wait for its group's
        # last copy can never stall input descriptor-gen (sync ring) or the
        # PSUM-freeing copies (scalar ring). Only 9 outs, so even a cold Q7
        # SWDGE keeps pace.

        # half-SB tail FIRST: smallest transfer starts the pipeline earliest
        xtt = xpool.tile([128, 4, 2, tail], F8, tag="xtail", name="xtt", bufs=1)
        nc.sync.dma_start(xtt[:], xtl)
        evt = evpool.tile([3, tail], F32, tag="evt", name="evt", bufs=1)
        do_sb(lambda a, h: xtt[:, a, :, :], evt, 0, 1)
        nc.gpsimd.dma_start(opl[:, nsb * 1024:nsb * 1024 + tail], evt[:])

        # then a single SB, then double-SB transfers (16 KB/partition lines),
        # then a single last SB (early drain)
        n2 = (nsb - 2) // 2
        groups = ([(0, 1)] + [(1 + 2 * i, 2) for i in range(n2)]
                  + [(1 + 2 * n2, 1)])
        assert sum(g for _, g in groups) == nsb
        for sb0, g in groups:
            xt = xpool.tile([128, g, 4, 2, 1024], F8, tag=f"x{g}",
                            name=f"x{g}", bufs=(3 if g == 1 else 5))
            nc.sync.dma_start(xt[:], xpk_v[:, sb0:sb0 + g, :, :, :])
            ev = evpool.tile([3, g * 1024], F32, tag=f"ev{g}", name=f"ev{g}",
                             bufs=3)
            for s2 in range(g):
                do_sb(lambda a, h, _xt=xt, _s2=s2:
                      _xt[:, _s2, a, :, 512 * h:512 * h + 512],
                      ev, 1024 * s2, 2)
            nc.gpsimd.dma_start(opl[:, 1024 * sb0:1024 * (sb0 + g)], ev[:])

    nc.compile()
    return nc


def _sigmoid_like_jax(x):
    # jax.nn.sigmoid: where(x >= 0, 1/(1+exp(-x)), exp(x)/(1+exp(x))) in f32
    x = x.astype(np.float32)
    pos = x >= 0
    ex_n = np.exp(np.where(pos, -x, x).astype(np.float32)).astype(np.float32)
    out = np.where(pos,
                   (np.float32(1.0) / (np.float32(1.0) + ex_n)).astype(np.float32),
                   (ex_n / (np.float32(1.0) + ex_n)).astype(np.float32))
    return out.astype(np.float32)


def _slide_max5(cm):
    # reference reduce_window: 5-window max, -inf edge padding. cm: [B, L]
    Bv, Lv = cm.shape
    ext = np.full((Bv, Lv + 4), -np.inf)
    ext[:, 2:Lv + 2] = cm
    return np.maximum.reduce([ext[:, k:k + Lv] for k in range(5)])


def _row_exact_full(xb, salb, Wc, bc, Ww, bw, Wo, bo):
    """Exact reference computation for one row (fallback path)."""
    c32 = (xb.astype(np.float64) @ Wc.astype(np.float64)).astype(np.float32)[:, 0]
    w32 = (xb.astype(np.float64) @ Ww.astype(np.float64)).astype(np.float32)[:, 0]
    o32 = (xb.astype(np.float64) @ Wo.astype(np.float64)).astype(np.float32)[:, 0]
    mask = (salb >= 0).astype(np.float32)
    cp = _sigmoid_like_jax(c32 + bc) * mask
    hm = _slide_max5(cp[None].astype(np.float64))[0]
    cpn = cp * (hm == cp.astype(np.float64)).astype(np.float32)
    order = np.lexsort((np.arange(L), -cpn.astype(np.float64)))[:TOPK]
    return order, cpn[order], (w32 + bw)[order], (o32 + bo)[order]


def _levels_fp8(W):
    """3 e4m3 levels, each scaled 16x vs previous. Returns [D,9] fp8 + scales.

    Column order: [V1c, V2c, V3c, V1w, V2w, V3w, V1o, V2o, V3o] so the three
    center-head levels land in PSUM partitions 0..2 (only those are shipped).
    """
    V1 = W.astype(E4M3)
    R1 = (W - V1.astype(np.float32)).astype(np.float32)
    V2 = (R1 * np.float32(16.0)).astype(E4M3)
    R2 = (R1 - V2.astype(np.float32) / np.float32(16.0)).astype(np.float32)
    V3 = (R2 * np.float32(256.0)).astype(E4M3)
    lv = np.concatenate([V1, V2, V3], axis=1)  # [D, 9] head-major levels
    lv = lv[:, [0, 3, 6, 1, 4, 7, 2, 5, 8]]   # -> level-major per head
    return lv, (1.0, 1.0 / 16.0, 1.0 / 256.0)


def kernel(x, saliency, Wc, bc, Ww, bw, Wo, bo):
    import os as _os
    use_f16 = bool(int(_os.environ.get("KERNEL_F16", "0")))

    x = np.asarray(x, dtype=np.float32)
    saliency = np.asarray(saliency, dtype=np.float32)
    Wc = np.asarray(Wc, dtype=np.float32)
    Ww = np.asarray(Ww, dtype=np.float32)
    Wo = np.asarray(Wo, dtype=np.float32)
    bc = np.float32(np.asarray(bc).reshape(-1)[0])
    bw = np.float32(np.asarray(bw).reshape(-1)[0])
    bo = np.float32(np.asarray(bo).reshape(-1)[0])

    W = np.concatenate([Wc, Ww, Wo], axis=1).astype(np.float32)  # [D, 3]
    if use_f16:
        DELTA = 4.0e-3
        K2_0 = 160
        bf = ml_dtypes.bfloat16
        Wh = W.astype(bf).astype(np.float32)
        Wm = (W - Wh).astype(bf).astype(np.float32)
        Wl = (W - Wh - Wm).astype(bf)
        lv = np.concatenate([Wh.astype(bf), Wm.astype(bf), Wl], axis=1)
        # device layout [128, 8, 9]: (k, c, m) = lv[128 c + k, m]
        sta_np = np.ascontiguousarray(
            lv.reshape(8, 128, 9).transpose(1, 0, 2)).astype(bf)
        scales = (1.0, 1.0, 1.0)
        plane_ix = (0, 3, 6)
        nsb16 = 17
        cap = nsb16 * 1024
        key = f"f16_{nsb16}"
        if key not in _NC_CACHE:
            _NC_CACHE[key] = _build_nc_fp16(nsb16)
    else:
        DELTA = 0.30
        K2_0 = 512
        lv, scales = _levels_fp8(W)
        W16 = np.zeros((D, 16), E4M3)
        W16[:, :9] = lv
        # device layout [128, 4, 2, 16]: (k, a, i, m) = W16[256 a + 128 i + k, m]
        sta_np = np.ascontiguousarray(
            W16.reshape(4, 2, 128, 16).transpose(2, 0, 1, 3))
        plane_ix = (0, 1, 2)
        cap = CAP
        key = f"f8_{NSB}_{TAIL}"
        if key not in _NC_CACHE:
            _NC_CACHE[key] = _build_nc_fp8(NSB, TAIL)
    nc = _NC_CACHE[key]

    mask_full = saliency >= 0
    in_maps, sels = [], []
    for cid in range(NCORES):
        r0 = cid * RPC
        xs = x[r0:r0 + RPC].reshape(NROW, D)
        sel = np.nonzero(mask_full[r0:r0 + RPC].reshape(NROW))[0]
        selc = sel[:cap]
        if use_f16:
            buf = np.zeros((cap, D), np.float16)
            buf[:len(selc)] = xs[selc].astype(np.float16)
            # [nsb16, 128, 8, 1024]: (s, k, c, j) = buf[1024 s + j, 128 c + k]
            xpk = np.ascontiguousarray(
                buf.reshape(nsb16, 1024, 8, 128).transpose(0, 3, 2, 1))
            in_maps.append({"xpk": xpk, "sta": sta_np})
        else:
            buf = np.zeros((cap, D), E4M3)
            buf[:len(selc)] = xs[selc].astype(E4M3)
            # [NSB, 128, 4, 2, 1024]: (s, k, a, i, j) = buf[1024 s + j, 256 a + 128 i + k]
            xpk = np.ascontiguousarray(
                buf[:NSB * 1024].reshape(NSB, 1024, 4, 2, 128)
                .transpose(0, 4, 2, 3, 1))
            # tail [128, 4, 2, TAIL]: (k, a, i, j) = buf[NSB*1024 + j, 256 a + 128 i + k]
            xtl = np.ascontiguousarray(
                buf[NSB * 1024:].reshape(TAIL, 4, 2, 128).transpose(3, 1, 2, 0))
            in_maps.append({"xpk": xpk, "xtl": xtl, "sta": sta_np})
        sels.append(sel)

    trace = bool(int(_os.environ.get("KERNEL_TRACE", "0")))
    res = run_bass_kernel_spmd(nc, in_maps, core_ids=list(range(NCORES)),
                               trace=trace)
    if trace and res.exec_time_ns is not None:
        print(f"HW exec time: {res.exec_time_ns} ns")
        kernel.last_exec_time_ns = res.exec_time_ns
        kernel.last_trace = res.instructions_and_trace

    Wc64, Ww64, Wo64 = (Wc.astype(np.float64), Ww.astype(np.float64),
                        Wo.astype(np.float64))

    # ---- host assembly: scatter compacted planes back to the full grid
    s0, s1, s2 = scales
    p0, p1, p2 = plane_ix
    c_hat = np.zeros((NCORES, NROW), np.float64)
    for cid in range(NCORES):
        pl = res.results[cid]["opl"].astype(np.float64)      # [3 or 9, cap]
        c_dev = pl[p0] * s0 + pl[p1] * s1 + pl[p2] * s2
        sel = sels[cid]
        selc = sel[:cap]
        c_hat[cid, selc] = c_dev[:len(selc)]
        if len(sel) > cap:  # overflow: exact host values (err 0)
            ov = sel[cap:]
            r0 = cid * RPC
            xo = x[r0:r0 + RPC].reshape(NROW, D)[ov].astype(np.float64)
            c_hat[cid, ov] = (xo @ Wc64)[:, 0]
    c_hat = c_hat.reshape(B, L)

    cm = c_hat.copy()
    cm[~mask_full] = -np.inf
    hm = _slide_max5(cm)
    pot = cm >= hm - (2 * DELTA + EPS)    # superset of exact NMS survivors
    kernel.last_margin = 0.0

    out = np.empty((B, TOPK, 3), np.float32)
    rows_fallback = 0
    for b in range(B):
        ok = False
        idx_pot = np.nonzero(pot[b])[0]
        idx_pot = idx_pot[np.isfinite(cm[b, idx_pot])]
        K2 = K2_0
        while K2 <= 4 * L and len(idx_pot) > 0:
            vp = cm[b, idx_pot]
            if len(idx_pot) > K2:
                top = np.argpartition(-vp, K2)[:K2]
                cutoff = vp[top].min()
                cand = idx_pot[top]
            else:
                cutoff = -np.inf
                cand = idx_pot
            # refine candidates + any window neighbor that could beat/tie one
            thr = np.full(L, np.inf)
            thr[cand] = cm[b, cand] - (2 * DELTA + EPS)
            thr_min = -_slide_max5(-thr[None])[0]
            need = np.zeros(L, bool)
            need[cand] = True
            need |= cm[b] >= thr_min
            R = np.nonzero(need)[0]

            xg = x[b, R].astype(np.float64)
            c32 = (xg @ Wc64).astype(np.float32)[:, 0]
            maskR = mask_full[b, R].astype(np.float32)
            cpR = _sigmoid_like_jax(c32 + bc) * maskR
            err = np.abs(c32.astype(np.float64) - c_hat[b, R]).max()
            kernel.last_margin = max(kernel.last_margin, float(err))
            if err > DELTA:
                break  # margin violated -> row fallback

            # exact NMS fate for candidates: cp_i == max(window cp);
            # unrefined window members are provably strictly below in f32.
            cp_map = np.zeros(L, np.float32)
            cp_map[R] = cpR
            refined = np.zeros(L, bool)
            refined[R] = True
            surv_idx, surv_cp = [], []
            for i in cand:
                lo, hi = max(0, i - 2), min(L, i + 3)
                win = np.arange(lo, hi)
                wmax = cp_map[win][refined[win]].max()
                if cp_map[i] == wmax:
                    surv_idx.append(i)
                    surv_cp.append(cp_map[i])
            surv_idx = np.asarray(surv_idx, np.int64)
            surv_cp = np.asarray(surv_cp, np.float32)
            if len(surv_idx) < TOPK:
                K2 *= 4
                continue
            order = np.lexsort((surv_idx, -surv_cp.astype(np.float64)))[:TOPK]
            inds_b = surv_idx[order]
            scores_b = surv_cp[order]
            # everything unrefined has c* <= cutoff + DELTA; need the 100th
            # winner's exact logit strictly above that by > EPS
            if np.isfinite(cutoff):
                if not (cm[b, inds_b[-1]] - DELTA > cutoff + DELTA + EPS):
                    K2 *= 4
                    continue
            ok = True
            break

        if not ok:
            inds_b, scores_b, winlog_b, offlog_b = _row_exact_full(
                x[b], saliency[b], Wc, bc, Ww, bw, Wo, bo)
            rows_fallback += 1
        else:
            xg = x[b, inds_b].astype(np.float64)
            winlog_b = (xg @ Ww64).astype(np.float32)[:, 0] + bw
            offlog_b = (xg @ Wo64).astype(np.float32)[:, 0] + bo

        indf = inds_b.astype(np.float32)
        win = np.clip(winlog_b.astype(np.float32), np.float32(0.0), None)
        off = offlog_b.astype(np.float32)
        center = np.clip((indf + off).astype(np.float32),
                         np.float32(0.0), np.float32(L - 1)).astype(np.float32)
        start = (np.clip((center - win * np.float32(0.5)).astype(np.float32),
                         np.float32(0.0), np.float32(L - 1))
                 * np.float32(UNIT)).astype(np.float32)
        end = (np.clip((center + win * np.float32(0.5)).astype(np.float32),
                       np.float32(0.0), np.float32(L - 1)) * np.float32(UNIT)
               + np.float32(UNIT)).astype(np.float32)
        out[b, :, 0] = start
        out[b, :, 1] = end
        out[b, :, 2] = scores_b
    kernel.rows_fallback = rows_fallback
    return out


# revision 33
# speedup vs baseline: 1.1452x; 1.1452x over previous
"""Trainium2 Bass kernel for nn_BoundaryHead_contrast (CenterNet-style 1D NMS head).

Strategy (8 NeuronCores, pure data parallel over batch):
  - Device is a pure matvec streamer over a COMPACTED stream: the host keeps
    only positions with saliency >= 0 (masked positions are provably inert:
    their cp is exactly 0, which can never beat an unmasked sigmoid and only
    yields score-0 survivors that cannot reach the top-100 while >= 100
    positive survivors exist - verified, with exact fallback).
  - fp8 mode (default): x is cast to e4m3 (1 B/elem) and packed
    [NSB, 128, 4, 2, 1024] for DoubleRow matmuls (K=256 per pass, 2 fp8
    elements per PE cell per cycle). The three [1024,1] heads ride in one
    [128, 4, 2, 16] fp8 stationary holding 3 heads x 3 e4m3 levels (scaled
    16x per level; host rescales and sums planes -> W exact to ~2^-12).
    HBM traffic: ~17 MiB/core vs 128 MiB f32-equivalent.
  - fp16 mode (KERNEL_F16=1): same pipeline at 2 B/elem without DoubleRow.
  - Host: scatters device center scores back to the full grid, applies the
    mask, runs 5-window NMS + top-k approximately, then refines every
    decision within a conservative margin DELTA by recomputing exact scores
    (f64 dot -> f32, replicating the reference's f32 elementwise ops and tie
    semantics in sigmoid space) for the borderline positions per row
    (~700/row at fp8 margins). Rows where any margin check fails fall back
    to exact host computation of the whole row.
"""

import numpy as np
import ml_dtypes
from contextlib import ExitStack

import concourse.bass as bass
import concourse.tile as tile
from concourse import bacc, mybir
from concourse.bass_utils import run_bass_kernel_spmd

B, L, D = 32, 8192, 1024
NCORES = 8
RPC = B // NCORES          # 4 rows per core
NROW = RPC * L             # 32768 positions per core
NSB = 16                   # full super-blocks of 1024 positions per core
TAIL = 512                 # extra half-super-block
CAP = NSB * 1024 + TAIL    # device capacity; unmasked ~16376 +- 90, max 16480
TOPK = 100
UNIT = 2
EPS = 2.0e-3               # slack so strict logit gaps survive f32 sigmoid

F16, BF16, F32 = mybir.dt.float16, mybir.dt.bfloat16, mybir.dt.float32
F8 = mybir.dt.float8e4
E4M3 = ml_dtypes.float8_e4m3fn

_NC_CACHE = {}


def _build_nc_fp16(nsb):
    nc = bacc.Bacc("TRN2", target_bir_lowering=False, debug=False)
    xpk = nc.dram_tensor("xpk", [nsb, 128, 8, 1024], F16, kind="ExternalInput").ap()
    sta = nc.dram_tensor("sta", [128, 8, 9], BF16, kind="ExternalInput").ap()
    opl = nc.dram_tensor("opl", [9, nsb * 1024], F32, kind="ExternalOutput").ap()
    xpk_v = xpk.rearrange("s k c j -> k s c j")

    with tile.TileContext(nc) as tc, ExitStack() as ctx:
        cpool = ctx.enter_context(tc.tile_pool(name="const", bufs=1))
        xpool = ctx.enter_context(tc.tile_pool(name="xin", bufs=5))
        pspool = ctx.enter_context(tc.tile_pool(name="ps", bufs=3, space="PSUM"))
        evpool = ctx.enter_context(tc.tile_pool(name="ev", bufs=4))

        sta_sb = cpool.tile([128, 8, 9], BF16)
        nc.scalar.dma_start(sta_sb[:], sta)

        for sb in range(nsb):
            xt = xpool.tile([128, 1, 8, 1024], F16, tag="x", name="xt")
            nc.sync.dma_start(xt[:], xpk_v[:, sb:sb + 1, :, :])
            pss = [pspool.tile([9, 512], F32, tag=f"ps{h}", name=f"ps{h}")
                   for h in range(2)]
            for c in range(8):
                for h in range(2):
                    nc.tensor.matmul(pss[h][:, :], sta_sb[:, c, :],
                                     xt[:, 0, c, 512 * h:512 * h + 512],
                                     start=(c == 0), stop=(c == 7),
                                     skip_group_check=True)
            for h in range(2):
                ev = evpool.tile([9, 512], F32, tag=f"ev{h}", name=f"ev{h}")
                nc.scalar.copy(ev[:], pss[h][:])
                o0 = 1024 * sb + 512 * h
                nc.scalar.dma_start(opl[:, o0:o0 + 512], ev[:])

    nc.compile()
    return nc


def _build_nc_fp8(nsb, tail):
    nc = bacc.Bacc("TRN2", target_bir_lowering=False, debug=False)
    xpk = nc.dram_tensor("xpk", [nsb, 128, 4, 2, 1024], F8,
                         kind="ExternalInput").ap()
    xtl = nc.dram_tensor("xtl", [128, 4, 2, tail], F8, kind="ExternalInput").ap()
    sta = nc.dram_tensor("sta", [128, 4, 2, 16], F8, kind="ExternalInput").ap()
    opl = nc.dram_tensor("opl", [3, nsb * 1024 + tail], F32,
                         kind="ExternalOutput").ap()
    xpk_v = xpk.rearrange("s k a i j -> k s a i j")
    DR = mybir.MatmulPerfMode.DoubleRow

    with tile.TileContext(nc) as tc, ExitStack() as ctx:
        cpool = ctx.enter_context(tc.tile_pool(name="const", bufs=1))
        xpool = ctx.enter_context(tc.tile_pool(name="xin", bufs=6))
        pspool = ctx.enter_context(tc.tile_pool(name="ps", bufs=3, space="PSUM"))
        evpool = ctx.enter_context(tc.tile_pool(name="ev", bufs=4))

        sta_sb = cpool.tile([128, 4, 2, 16], F8)
        nc.scalar.dma_start(sta_sb[:], sta)

        # half-SB tail FIRST: the smallest transfer starts the pipeline
        # earliest; output scheme identical to the per-SB steady state
        xtt = xpool.tile([128, 4, 2, tail], F8, tag="xtail", name="xtt", bufs=1)
        nc.sync.dma_start(xtt[:], xtl)
        pst = pspool.tile([16, 512], F32, tag="ps0", name="ps0")
        for a in range(4):
            nc.tensor.matmul(pst[:, :], sta_sb[:, a, :, :], xtt[:, a, :, :],
                             start=(a == 0), stop=(a == 3),
                             perf_mode=DR, skip_group_check=True)
        evt = evpool.tile([3, 512], F32, tag="ev0", name="ev0")
        nc.scalar.copy(evt[:], pst[0:3, :])
        nc.gpsimd.dma_start(opl[:, nsb * 1024:nsb * 1024 + tail], evt[:])

        # group input DMAs: single SB first and last, double-SB transfers
        # in between (16 KB/partition lines halve descriptor count)
        n2 = (nsb - 2) // 2
        groups = ([(0, 1)] + [(1 + 2 * i, 2) for i in range(n2)]
                  + [(1 + 2 * n2, 1)])
        assert sum(g for _, g in groups) == nsb
        for sb0, g in groups:
            xt = xpool.tile([128, g, 4, 2, 1024], F8, tag=f"x{g}",
                            name=f"x{g}", bufs=(3 if g == 1 else 6))
            nc.sync.dma_start(xt[:], xpk_v[:, sb0:sb0 + g, :, :, :])
            for s2 in range(g):
                pss = [pspool.tile([16, 512], F32, tag=f"ps{h}", name=f"ps{h}")
                       for h in range(2)]
                for h in range(2):
                    for a in range(4):
                        nc.tensor.matmul(pss[h][:, :], sta_sb[:, a, :, :],
                                         xt[:, s2, a, :, 512 * h:512 * h + 512],
                                         start=(a == 0), stop=(a == 3),
                                         perf_mode=DR, skip_group_check=True)
                for h in range(2):
                    ev = evpool.tile([3, 512], F32, tag=f"ev{h}", name=f"ev{h}")
                    nc.scalar.copy(ev[:], pss[h][0:3, :])
                    o0 = 1024 * (sb0 + s2) + 512 * h
                    nc.gpsimd.dma_start(opl[:, o0:o0 + 512], ev[:])

    nc.compile()
    return nc


def _sigmoid_like_jax(x):
    # jax.nn.sigmoid: where(x >= 0, 1/(1+exp(-x)), exp(x)/(1+exp(x))) in f32
    x = x.astype(np.float32)
    pos = x >= 0
    ex_n = np.exp(np.where(pos, -x, x).astype(np.float32)).astype(np.float32)
    out = np.where(pos,
                   (np.float32(1.0) / (np.float32(1.0) + ex_n)).astype(np.float32),
                   (ex_n / (np.float32(1.0) + ex_n)).astype(np.float32))
    return out.astype(np.float32)


def _slide_max5(cm):
    # reference reduce_window: 5-window max, -inf edge padding. cm: [B, L]
    Bv, Lv = cm.shape
    ext = np.full((Bv, Lv + 4), -np.inf)
    ext[:, 2:Lv + 2] = cm
    return np.maximum.reduce([ext[:, k:k + Lv] for k in range(5)])


def _row_exact_full(xb, salb, Wc, bc, Ww, bw, Wo, bo):
    """Exact reference computation for one row (fallback path)."""
    c32 = (xb.astype(np.float64) @ Wc.astype(np.float64)).astype(np.float32)[:, 0]
    w32 = (xb.astype(np.float64) @ Ww.astype(np.float64)).astype(np.float32)[:, 0]
    o32 = (xb.astype(np.float64) @ Wo.astype(np.float64)).astype(np.float32)[:, 0]
    mask = (salb >= 0).astype(np.float32)
    cp = _sigmoid_like_jax(c32 + bc) * mask
    hm = _slide_max5(cp[None].astype(np.float64))[0]
    cpn = cp * (hm == cp.astype(np.float64)).astype(np.float32)
    order = np.lexsort((np.arange(L), -cpn.astype(np.float64)))[:TOPK]
    return order, cpn[order], (w32 + bw)[order], (o32 + bo)[order]


def _levels_fp8(W):
    """3 e4m3 levels, each scaled 16x vs previous. Returns [D,9] fp8 + scales.

    Column order: [V1c, V2c, V3c, V1w, V2w, V3w, V1o, V2o, V3o] so the three
    center-head levels land in PSUM partitions 0..2 (only those are shipped).
    """
    V1 = W.astype(E4M3)
    R1 = (W - V1.astype(np.float32)).astype(np.float32)
    V2 = (R1 * np.float32(16.0)).astype(E4M3)
    R2 = (R1 - V2.astype(np.float32) / np.float32(16.0)).astype(np.float32)
    V3 = (R2 * np.float32(256.0)).astype(E4M3)
    lv = np.concatenate([V1, V2, V3], axis=1)  # [D, 9] head-major levels
    lv = lv[:, [0, 3, 6, 1, 4, 7, 2, 5, 8]]   # -> level-major per head
    return lv, (1.0, 1.0 / 16.0, 1.0 / 256.0)


def kernel(x, saliency, Wc, bc, Ww, bw, Wo, bo):
    import os as _os
    use_f16 = bool(int(_os.environ.get("KERNEL_F16", "0")))

    x = np.asarray(x, dtype=np.float32)
    saliency = np.asarray(saliency, dtype=np.float32)
    Wc = np.asarray(Wc, dtype=np.float32)
    Ww = np.asarray(Ww, dtype=np.float32)
    Wo = np.asarray(Wo, dtype=np.float32)
    bc = np.float32(np.asarray(bc).reshape(-1)[0])
    bw = np.float32(np.asarray(bw).reshape(-1)[0])
    bo = np.float32(np.asarray(bo).reshape(-1)[0])

    W = np.concatenate([Wc, Ww, Wo], axis=1).astype(np.float32)  # [D, 3]
    if use_f16:
        DELTA = 4.0e-3
        K2_0 = 160
        bf = ml_dtypes.bfloat16
        Wh = W.astype(bf).astype(np.float32)
        Wm = (W - Wh).astype(bf).astype(np.float32)
        Wl = (W - Wh - Wm).astype(bf)
        lv = np.concatenate([Wh.astype(bf), Wm.astype(bf), Wl], axis=1)
        # device layout [128, 8, 9]: (k, c, m) = lv[128 c + k, m]
        sta_np = np.ascontiguousarray(
            lv.reshape(8, 128, 9).transpose(1, 0, 2)).astype(bf)
        scales = (1.0, 1.0, 1.0)
        plane_ix = (0, 3, 6)
        nsb16 = 17
        cap = nsb16 * 1024
        key = f"f16_{nsb16}"
        if key not in _NC_CACHE:
            _NC_CACHE[key] = _build_nc_fp16(nsb16)
    else:
        DELTA = 0.30
        K2_0 = 512
        lv, scales = _levels_fp8(W)
        W16 = np.zeros((D, 16), E4M3)
        W16[:, :9] = lv
        # device layout [128, 4, 2, 16]: (k, a, i, m) = W16[256 a + 128 i + k, m]
        sta_np = np.ascontiguousarray(
            W16.reshape(4, 2, 128, 16).transpose(2, 0, 1, 3))
        plane_ix = (0, 1, 2)
        cap = CAP
        key = f"f8_{NSB}_{TAIL}"
        if key not in _NC_CACHE:
            _NC_CACHE[key] = _build_nc_fp8(NSB, TAIL)
    nc = _NC_CACHE[key]

    mask_full = saliency >= 0
    in_maps, sels = [], []
    for cid in range(NCORES):
        r0 = cid * RPC
        xs = x[r0:r0 + RPC].reshape(NROW, D)
        sel = np.nonzero(mask_full[r0:r0 + RPC].reshape(NROW))[0]
        selc = sel[:cap]
        if use_f16:
            buf = np.zeros((cap, D), np.float16)
            buf[:len(selc)] = xs[selc].astype(np.float16)
            # [nsb16, 128, 8, 1024]: (s, k, c, j) = buf[1024 s + j, 128 c + k]
            xpk = np.ascontiguousarray(
                buf.reshape(nsb16, 1024, 8, 128).transpose(0, 3, 2, 1))
            in_maps.append({"xpk": xpk, "sta": sta_np})
        else:
            buf = np.zeros((cap, D), E4M3)
            buf[:len(selc)] = xs[selc].astype(E4M3)
            # [NSB, 128, 4, 2, 1024]: (s, k, a, i, j) = buf[1024 s + j, 256 a + 128 i + k]
            xpk = np.ascontiguousarray(
                buf[:NSB * 1024].reshape(NSB, 1024, 4, 2, 128)
                .transpose(0, 4, 2, 3, 1))
            # tail [128, 4, 2, TAIL]: (k, a, i, j) = buf[NSB*1024 + j, 256 a + 128 i + k]
            xtl = np.ascontiguousarray(
                buf[NSB * 1024:].reshape(TAIL, 4, 2, 128).transpose(3, 1, 2, 0))
            in_maps.append({"xpk": xpk, "xtl": xtl, "sta": sta_np})
        sels.append(sel)

    trace = bool(int(_os.environ.get("KERNEL_TRACE", "0")))
    res = run_bass_kernel_spmd(nc, in_maps, core_ids=list(range(NCORES)),
                               trace=trace)
    if trace and res.exec_time_ns is not None:
        print(f"HW exec time: {res.exec_time_ns} ns")
        kernel.last_exec_time_ns = res.exec_time_ns
        kernel.last_trace = res.instructions_and_trace

    Wc64, Ww64, Wo64 = (Wc.astype(np.float64), Ww.astype(np.float64),
                        Wo.astype(np.float64))

    # ---- host assembly: scatter compacted planes back to the full grid
    s0, s1, s2 = scales
    p0, p1, p2 = plane_ix
    c_hat = np.zeros((NCORES, NROW), np.float64)
    for cid in range(NCORES):
        pl = res.results[cid]["opl"].astype(np.float64)      # [3 or 9, cap]
        c_dev = pl[p0] * s0 + pl[p1] * s1 + pl[p2] * s2
        sel = sels[cid]
        selc = sel[:cap]
        c_hat[cid, selc] = c_dev[:len(selc)]
        if len(sel) > cap:  # overflow: exact host values (err 0)
            ov = sel[cap:]
            r0 = cid * RPC
            xo = x[r0:r0 + RPC].reshape(NROW, D)[ov].astype(np.float64)
            c_hat[cid, ov] = (xo @ Wc64)[:, 0]
    c_hat = c_hat.reshape(B, L)

    cm = c_hat.copy()
    cm[~mask_full] = -np.inf
    hm = _slide_max5(cm)
    pot = cm >= hm - (2 * DELTA + EPS)    # superset of exact NMS survivors
    kernel.last_margin = 0.0

    out = np.empty((B, TOPK, 3), np.float32)
    rows_fallback = 0
    for b in range(B):
        ok = False
        idx_pot = np.nonzero(pot[b])[0]
        idx_pot = idx_pot[np.isfinite(cm[b, idx_pot])]
        K2 = K2_0
        while K2 <= 4 * L and len(idx_pot) > 0:
            vp = cm[b, idx_pot]
            if len(idx_pot) > K2:
                top = np.argpartition(-vp, K2)[:K2]
                cutoff = vp[top].min()
                cand = idx_pot[top]
            else:
                cutoff = -np.inf
                cand = idx_pot
            # refine candidates + any window neighbor that could beat/tie one
            thr = np.full(L, np.inf)
            thr[cand] = cm[b, cand] - (2 * DELTA + EPS)
            thr_min = -_slide_max5(-thr[None])[0]
            need = np.zeros(L, bool)
            need[cand] = True
            need |= cm[b] >= thr_min
            R = np.nonzero(need)[0]

            xg = x[b, R].astype(np.float64)
            c32 = (xg @ Wc64).astype(np.float32)[:, 0]
            maskR = mask_full[b, R].astype(np.float32)
            cpR = _sigmoid_like_jax(c32 + bc) * maskR
            err = np.abs(c32.astype(np.float64) - c_hat[b, R]).max()
            kernel.last_margin = max(kernel.last_margin, float(err))
            if err > DELTA:
                break  # margin violated -> row fallback

            # exact NMS fate for candidates: cp_i == max(window cp);
            # unrefined window members are provably strictly below in f32.
            cp_map = np.zeros(L, np.float32)
            cp_map[R] = cpR
            refined = np.zeros(L, bool)
            refined[R] = True
            surv_idx, surv_cp = [], []
            for i in cand:
                lo, hi = max(0, i - 2), min(L, i + 3)
                win = np.arange(lo, hi)
                wmax = cp_map[win][refined[win]].max()
                if cp_map[i] == wmax:
                    surv_idx.append(i)
                    surv_cp.append(cp_map[i])
            surv_idx = np.asarray(surv_idx, np.int64)
            surv_cp = np.asarray(surv_cp, np.float32)
            if len(surv_idx) < TOPK:
                K2 *= 4
                continue
            order = np.lexsort((surv_idx, -surv_cp.astype(np.float64)))[:TOPK]
            inds_b = surv_idx[order]
            scores_b = surv_cp[order]
            # everything unrefined has c* <= cutoff + DELTA; need the 100th
            # winner's exact logit strictly above that by > EPS
            if np.isfinite(cutoff):
                if not (cm[b, inds_b[-1]] - DELTA > cutoff + DELTA + EPS):
                    K2 *= 4
                    continue
            ok = True
            break

        if not ok:
            inds_b, scores_b, winlog_b, offlog_b = _row_exact_full(
                x[b], saliency[b], Wc, bc, Ww, bw, Wo, bo)
            rows_fallback += 1
        else:
            xg = x[b, inds_b].astype(np.float64)
            winlog_b = (xg @ Ww64).astype(np.float32)[:, 0] + bw
            offlog_b = (xg @ Wo64).astype(np.float32)[:, 0] + bo

        indf = inds_b.astype(np.float32)
        win = np.clip(winlog_b.astype(np.float32), np.float32(0.0), None)
        off = offlog_b.astype(np.float32)
        center = np.clip((indf + off).astype(np.float32),
                         np.float32(0.0), np.float32(L - 1)).astype(np.float32)
        start = (np.clip((center - win * np.float32(0.5)).astype(np.float32),
                         np.float32(0.0), np.float32(L - 1))
                 * np.float32(UNIT)).astype(np.float32)
        end = (np.clip((center + win * np.float32(0.5)).astype(np.float32),
                       np.float32(0.0), np.float32(L - 1)) * np.float32(UNIT)
               + np.float32(UNIT)).astype(np.float32)
        out[b, :, 0] = start
        out[b, :, 1] = end
        out[b, :, 2] = scores_b
    kernel.rows_fallback = rows_fallback
    return out
